# revision 7
# baseline (speedup 1.0000x reference)
"""BaoNet GNN message-passing kernel for 8 Trainium2 NeuronCores.

Strategy (one uniform SPMD program, all per-device variability in data):
- Partition graphs into 8 contiguous blocks of 128 graphs; each device owns
  the nodes/edges whose dst falls in its slice (dst-sharding).
- Node features h live in a replicated HBM table [8*S, 128ch] bf16 (64 real
  channels), rebuilt every layer via AllGather.
- Message pass per layer: edges are placed into fixed "slots": for each
  (window of 128 dst nodes, half-window of 64, src-quarter q) there are B
  blocks of 128 slots. h[src] rows are fetched with dma_gather (int16 local
  indices into the 2S-row quarter of the table); a host-built one-hot matrix
  O [128 slots, 64 dst-cols] bf16 turns PE matmuls G.T @ O into the
  segment-sum: msgT[c, dstcol] accumulated in PSUM. Pad slots have O == 0.
- h update: hT_new = leaky(Wself.T @ hT + Wnbr.T @ msgT + b) on PE, kept
  transposed [64, S] f32 in SBUF; transposed back per window via PE for the
  table staging (bf16) and, after the last layer, for mean-pooling via a
  host-built pooling matrix P; final 3-layer MLP on PE.
"""
import sys
import os

sys.path.insert(0, "/opt/trn_rl_repo")

import numpy as np
import ml_dtypes
from contextlib import ExitStack

# ---------------- problem constants (hardcoded per spec) ----------------
N_NODES = 100000
N_EDGES = 3200000
N_GRAPHS = 1024
IN_DIM, HID, OUT_DIM = 13, 64, 72
N_LAYERS = 4
N_CORES = 8
GPD = N_GRAPHS // N_CORES          # graphs per device (128)
WGN = 4                            # windows per window-group / psum tile

BF16 = ml_dtypes.bfloat16

_CACHE = {}


# ======================= host-side preprocessing =======================

def _prep(Vnode, Vedge, y):
    src = np.asarray(Vedge[0], dtype=np.int64)
    dst = np.asarray(Vedge[1], dtype=np.int64)
    y = np.asarray(y, dtype=np.int64)
    Vnode = np.asarray(Vnode, dtype=np.float32)

    gstart = np.searchsorted(y, np.arange(0, N_GRAPHS + 1, GPD))
    sizes = np.diff(gstart)
    S = int(np.ceil((sizes.max() + 128) / (128 * WGN)) * 128 * WGN)
    NW = S // 128
    NQ = max(1, N_CORES // 2)
    QSPAN = N_CORES * S // NQ
    assert QSPAN <= 32768, f"quarter span {QSPAN} exceeds int16 reach"

    # global table row of each node
    nid = np.arange(N_NODES)
    dev_of_node = np.searchsorted(gstart, nid, side="right") - 1
    srow = dev_of_node * S + (nid - gstart[dev_of_node])

    e_dev = np.searchsorted(gstart, dst, side="right") - 1
    e_srow = srow[src]
    e_q = e_srow // QSPAN                       # src quarter 0..3
    e_sloc = (e_srow - e_q * QSPAN).astype(np.int64)   # local idx < QSPAN
    e_local = dst - gstart[e_dev]               # local dst
    e_w = e_local // 128                        # window
    e_h = (e_local // 64) % 2                   # half window
    e_col = e_local - e_w * 128 - e_h * 64      # one-hot column 0..63

    # B = max blocks needed for any (dev, q, w, h) cell
    cell = ((e_dev * NQ + e_q) * NW + e_w) * 2 + e_h
    counts = np.bincount(cell, minlength=N_CORES * NQ * NW * 2)
    B = max(2, int(np.ceil(counts.max() / 128)))

    # slot layout: chunk (wg, q) has 8*B blocks of 128 slots
    #   block index within chunk: (w % WGN) * 2B + h * B + k
    order = np.lexsort((e_local, e_h, e_w, e_q, e_dev))
    so_cell = cell[order]
    # sequence number within cell
    cum = np.concatenate([[0], np.cumsum(counts)])
    k_in_cell = np.arange(len(order)) - cum[so_cell]

    sd = e_dev[order]
    sq = e_q[order]
    sw = e_w[order]
    sh = e_h[order]
    scol = e_col[order]
    sloc = e_sloc[order]

    nwg = NW // WGN
    chunk_slots = 8 * B * 128                    # slots per (wg, q) chunk
    chunk_of = (sw // WGN) * NQ + sq             # chunk within device
    blk_in_chunk = (sw % WGN) * 2 * B + sh * B + (k_in_cell // 128)
    pos = chunk_of * chunk_slots + blk_in_chunk * 128 + (k_in_cell % 128)

    nchunks = nwg * NQ
    tot_slots = nchunks * chunk_slots
    nblocks = nchunks * 8 * B

    idxs = np.zeros((N_CORES, tot_slots), np.int16)
    Obuf = np.zeros((N_CORES, 128, nblocks * 64), np.uint16)
    one_bf16 = np.float32(1.0).astype(BF16).view(np.uint16)
    for d in range(N_CORES):
        m = sd == d
        p = pos[m]
        idxs[d, p] = sloc[m].astype(np.int16)
        blk = p // 128
        row = p % 128
        Obuf[d, row, blk * 64 + scol[m]] = one_bf16
    # wrap idx streams: slot i of chunk -> partition i%16, col i//16, x8 rep
    idxs = idxs.reshape(N_CORES, nchunks, chunk_slots // 16, 16)
    idxs = np.ascontiguousarray(idxs.transpose(0, 1, 3, 2))  # [D, nch, 16, cs/16]
    idxs = np.tile(idxs, (1, 1, 8, 1)).reshape(N_CORES, nchunks, 128, chunk_slots // 16)
    idxs = np.ascontiguousarray(idxs.transpose(0, 2, 1, 3)).reshape(
        N_CORES, 128, nchunks * chunk_slots // 16)

    # per-device padded Vnode slices, pooling matrices, inverse counts
    vnode_dev = np.zeros((N_CORES, S, IN_DIM), np.float32)
    P = np.zeros((N_CORES, S, GPD), np.float32)
    invcnt = np.ones((N_CORES, GPD, 1), np.float32)
    for d in range(N_CORES):
        L = sizes[d]
        vnode_dev[d, :L] = Vnode[gstart[d]:gstart[d + 1]]
        gl = y[gstart[d]:gstart[d + 1]] - d * GPD
        P[d, np.arange(L), gl] = 1.0
        cnt = np.bincount(gl, minlength=GPD).astype(np.float32)
        invcnt[d, :, 0] = 1.0 / np.maximum(cnt, 1.0)

    return dict(S=S, NW=NW, B=B, nwg=nwg, NQ=NQ, nchunks=nchunks,
                chunk_slots=chunk_slots, nblocks=nblocks,
                idxs=idxs, Obuf=Obuf.view(BF16), vnode_dev=vnode_dev,
                P=P, invcnt=invcnt)


# ======================= bass program =======================

def _build(cfg):
    import concourse.bass as bass
    import concourse.tile as tile
    from concourse import bacc, mybir
    from concourse.masks import make_identity

    S, NW, B, nwg = cfg["S"], cfg["NW"], cfg["B"], cfg["nwg"]
    chunk_slots, nblocks = cfg["chunk_slots"], cfg["nblocks"]
    NQ = cfg["NQ"]
    QSPAN = N_CORES * S // NQ
    f32, bf16, i16 = mybir.dt.float32, mybir.dt.bfloat16, mybir.dt.int16
    CPB = chunk_slots // 128        # blocks per chunk (8B)
    CIDX = chunk_slots // 16        # idx cols per chunk

    nc = bacc.Bacc("TRN2", target_bir_lowering=False, debug=False,
                   enable_asserts=False, num_devices=N_CORES)
    # ---- I/O ----
    t_vn = nc.dram_tensor("vnode", [S, IN_DIM], f32, kind="ExternalInput").ap()
    t_idx = nc.dram_tensor("idxs", [128, nwg * NQ * CIDX], i16, kind="ExternalInput").ap()
    t_O = nc.dram_tensor("obuf", [128, nblocks * 64], bf16, kind="ExternalInput").ap()
    t_P = nc.dram_tensor("pmat", [S, GPD], f32, kind="ExternalInput").ap()
    t_ic = nc.dram_tensor("invcnt", [GPD, 1], f32, kind="ExternalInput").ap()
    t_Win = nc.dram_tensor("W_in", [IN_DIM, HID], f32, kind="ExternalInput").ap()
    t_bin = nc.dram_tensor("b_in", [HID, 1], f32, kind="ExternalInput").ap()
    t_Ws = nc.dram_tensor("Wself", [N_LAYERS, HID, HID], f32, kind="ExternalInput").ap()
    t_Wn = nc.dram_tensor("Wnbr", [N_LAYERS, HID, HID], f32, kind="ExternalInput").ap()
    t_bl = nc.dram_tensor("bl", [N_LAYERS, HID, 1], f32, kind="ExternalInput").ap()
    t_Wo = nc.dram_tensor("Wout", [HID, OUT_DIM], f32, kind="ExternalInput").ap()
    t_bo = nc.dram_tensor("bout", [OUT_DIM, 1], f32, kind="ExternalInput").ap()
    t_W1 = nc.dram_tensor("W1", [OUT_DIM, 36], f32, kind="ExternalInput").ap()
    t_b1 = nc.dram_tensor("b1", [36, 1], f32, kind="ExternalInput").ap()
    t_W2 = nc.dram_tensor("W2", [36, 1], f32, kind="ExternalInput").ap()
    t_b2 = nc.dram_tensor("b2", [1, 1], f32, kind="ExternalInput").ap()
    t_out = nc.dram_tensor("out", [1, GPD], f32, kind="ExternalOutput").ap()

    with tile.TileContext(nc) as tc, ExitStack() as ctx:
        cpool = ctx.enter_context(tc.tile_pool(name="const", bufs=1))
        hpool = ctx.enter_context(tc.tile_pool(name="h", bufs=1))
        gpool = ctx.enter_context(tc.tile_pool(name="g", bufs=3))
        opool = ctx.enter_context(tc.tile_pool(name="o", bufs=3))
        ipool = ctx.enter_context(tc.tile_pool(name="idx", bufs=3))
        mpool = ctx.enter_context(tc.tile_pool(name="msg", bufs=3))
        wpool = ctx.enter_context(tc.tile_pool(name="work", bufs=3))
        ppool = ctx.enter_context(tc.tile_pool(name="pp", bufs=2))
        pspool = ctx.enter_context(tc.tile_pool(name="ps", bufs=2, space="PSUM"))
        ps1pool = ctx.enter_context(tc.tile_pool(name="ps1", bufs=4, space="PSUM"))
        pgpool = ctx.enter_context(tc.tile_pool(name="pg", bufs=1, space="PSUM"))
        dpool = ctx.enter_context(tc.tile_pool(name="dram", bufs=1, space="DRAM"))

        # persistent tiles
        ident = cpool.tile([128, 128], f32, tag="ident")
        make_identity(nc, ident[:])
        staging = cpool.tile([128, NW, 128], bf16, tag="staging")
        nc.vector.memset(staging[:], 0.0)
        hT = [hpool.tile([HID, S], f32, tag=f"hT{i}", name=f"hT{i}")
              for i in range(2)]
        ag_in = dpool.tile([S, 128], bf16, tag="agin")
        table = dpool.tile([N_CORES * S, 128], bf16, tag="table")

        def load_const(t, shape, dtype=f32, tag=None):
            tl = cpool.tile(shape, dtype, tag=tag or t.tensor.name)
            nc.sync.dma_start(tl[:], t)
            return tl

        Win = load_const(t_Win, [IN_DIM, HID])
        binT = load_const(t_bin, [HID, 1])
        Ws, Wn, bl = [], [], []
        for l in range(N_LAYERS):
            wtile = cpool.tile([HID, HID], f32, tag=f"Ws{l}", name=f"Ws{l}")
            nc.sync.dma_start(wtile[:], t_Ws[l])
            Ws.append(wtile)
            ntile = cpool.tile([HID, HID], f32, tag=f"Wn{l}", name=f"Wn{l}")
            nc.sync.dma_start(ntile[:], t_Wn[l])
            Wn.append(ntile)
            btile = cpool.tile([HID, 1], f32, tag=f"bl{l}", name=f"bl{l}")
            nc.sync.dma_start(btile[:], t_bl[l])
            bl.append(btile)
        Wo = load_const(t_Wo, [HID, OUT_DIM])
        bo = load_const(t_bo, [OUT_DIM, 1])
        W1 = load_const(t_W1, [OUT_DIM, 36])
        b1 = load_const(t_b1, [36, 1])
        W2 = load_const(t_W2, [36, 1])
        b2 = load_const(t_b2, [1, 1])
        icnt = load_const(t_ic, [GPD, 1])

        def leaky_from_psum(dst_ap, psum_ap, bias_ap):
            # dst = leaky_relu(psum + bias), via t = psum+bias; max(t, .01t)
            t = wpool.tile([HID, 128], f32, tag="lk_t")
            nc.scalar.activation(t[:], psum_ap, mybir.ActivationFunctionType.Identity,
                                 bias=bias_ap)
            m = wpool.tile([HID, 128], f32, tag="lk_m")
            nc.vector.tensor_scalar_mul(m[:], t[:], 0.01)
            nc.vector.tensor_tensor(out=dst_ap, in0=t[:], in1=m[:],
                                    op=mybir.AluOpType.max)

        def stage_window(h_src, w):
            # transpose hT window [64,128] -> [128,64], write staging bf16
            pt = ps1pool.tile([128, HID], f32, tag="pstmp")
            nc.tensor.transpose(pt[:], h_src[:, w * 128:(w + 1) * 128], ident[:HID, :HID])
            nc.scalar.activation(staging[:, w, 0:HID], pt[:],
                                 mybir.ActivationFunctionType.Copy)

        # ---------------- h0 ----------------
        for w in range(NW):
            vt = wpool.tile([128, IN_DIM], f32, tag="vt")
            nc.sync.dma_start(vt[:], t_vn[w * 128:(w + 1) * 128, :])
            pvt = ps1pool.tile([IN_DIM, 128], f32, tag="pstmp")
            nc.tensor.transpose(pvt[:], vt[:], ident[:])
            vT = wpool.tile([IN_DIM, 128], f32, tag="vT")
            nc.scalar.activation(vT[:], pvt[:], mybir.ActivationFunctionType.Copy)
            ph = ps1pool.tile([HID, 128], f32, tag="pstmp")
            nc.tensor.matmul(out=ph[:], lhsT=Win[:], rhs=vT[:], start=True, stop=True)
            leaky_from_psum(hT[0][:, w * 128:(w + 1) * 128], ph[:], binT[:])
            stage_window(hT[0], w)
        nc.sync.dma_start(
            ag_in.rearrange("(w p) c -> p w c", p=128)[:], staging[:])
        nc.gpsimd.collective_compute(
            "AllGather", mybir.AluOpType.bypass,
            replica_groups=[list(range(N_CORES))],
            ins=[ag_in.opt()], outs=[table.opt()])

        # ---------------- layers ----------------
        pgs = pgpool.tile([GPD, HID], f32, tag="pool_ps")
        for l in range(N_LAYERS):
            hsrc, hdst = hT[l % 2], hT[(l + 1) % 2]
            for wg in range(nwg):
                psw = pspool.tile([HID, WGN * 128], f32, tag="psw")
                nc.vector.memset(psw[:], 0.0)
                for q in range(NQ):
                    ci = wg * NQ + q
                    it = ipool.tile([128, CIDX], i16, tag="it")
                    nc.sync.dma_start(it[:], t_idx[:, ci * CIDX:(ci + 1) * CIDX])
                    ot = opool.tile([128, CPB * 64], bf16, tag="ot")
                    nc.sync.dma_start(
                        ot[:], t_O[:, ci * CPB * 64:(ci + 1) * CPB * 64])
                    g = gpool.tile([128, CPB, 128], bf16, tag="g")
                    nc.gpsimd.dma_gather(
                        out_ap=g[:], in_ap=table[q * QSPAN:(q + 1) * QSPAN, :],
                        idxs_ap=it[:], num_idxs=chunk_slots,
                        num_idxs_reg=chunk_slots, elem_size=128,
                        single_packet=False)
                    for b in range(CPB):
                        wi = b // (2 * B)          # window in group
                        hi = (b // B) % 2          # half
                        nc.tensor.matmul(
                            out=psw[:, wi * 128 + hi * 64: wi * 128 + hi * 64 + 64],
                            lhsT=g[:, b, 0:HID],
                            rhs=ot[:, b * 64:(b + 1) * 64],
                            start=False, stop=(q == NQ - 1 and b == CPB - 1),
                            skip_group_check=True)
                for wi in range(WGN):
                    w = wg * WGN + wi
                    msgT = mpool.tile([HID, 128], f32, tag="msgT")
                    nc.scalar.activation(msgT[:], psw[:, wi * 128:(wi + 1) * 128],
                                         mybir.ActivationFunctionType.Copy)
                    pu = ps1pool.tile([HID, 128], f32, tag="pstmp")
                    nc.tensor.matmul(out=pu[:], lhsT=Ws[l][:], rhs=hsrc[:, w * 128:(w + 1) * 128],
                                     start=True, stop=False)
                    nc.tensor.matmul(out=pu[:], lhsT=Wn[l][:], rhs=msgT[:],
                                     start=False, stop=True)
                    leaky_from_psum(hdst[:, w * 128:(w + 1) * 128], pu[:], bl[l][:])
                    if l < N_LAYERS - 1:
                        stage_window(hdst, w)
                    else:
                        # pooling contribution of this window
                        pt = ps1pool.tile([128, HID], f32, tag="pstmp")
                        nc.tensor.transpose(pt[:], hdst[:, w * 128:(w + 1) * 128],
                                            ident[:HID, :HID])
                        rowt = wpool.tile([128, HID], f32, tag="rowt")
                        nc.scalar.activation(rowt[:], pt[:],
                                             mybir.ActivationFunctionType.Copy)
                        pw = ppool.tile([128, GPD], f32, tag="pw")
                        nc.sync.dma_start(pw[:], t_P[w * 128:(w + 1) * 128, :])
                        nc.tensor.matmul(out=pgs[:], lhsT=pw[:], rhs=rowt[:],
                                         start=(w == 0), stop=(w == NW - 1),
                                         skip_group_check=True)
            if l < N_LAYERS - 1:
                nc.sync.dma_start(
                    ag_in.rearrange("(w p) c -> p w c", p=128)[:], staging[:])
                nc.gpsimd.collective_compute(
                    "AllGather", mybir.AluOpType.bypass,
                    replica_groups=[list(range(N_CORES))],
                    ins=[ag_in.opt()], outs=[table.opt()])

        # ---------------- pooling mean + MLP ----------------
        pooled = cpool.tile([GPD, HID], f32, tag="pooled")
        nc.vector.tensor_scalar(out=pooled[:], in0=pgs[:], scalar1=icnt[:],
                                scalar2=None, op0=mybir.AluOpType.mult)
        ptp = ps1pool.tile([HID, GPD], f32, tag="pstmp")
        nc.tensor.transpose(ptp[:], pooled[:], ident[:GPD, :GPD])
        pooledT = cpool.tile([HID, GPD], f32, tag="pooledT")
        nc.scalar.activation(pooledT[:], ptp[:], mybir.ActivationFunctionType.Copy)

        px1 = ps1pool.tile([OUT_DIM, GPD], f32, tag="pstmp")
        nc.tensor.matmul(out=px1[:], lhsT=Wo[:], rhs=pooledT[:], start=True, stop=True)
        x1 = cpool.tile([OUT_DIM, GPD], f32, tag="x1")
        nc.scalar.activation(x1[:], px1[:], mybir.ActivationFunctionType.Identity,
                             bias=bo[:])
        px2 = ps1pool.tile([36, GPD], f32, tag="pstmp")
        nc.tensor.matmul(out=px2[:], lhsT=W1[:], rhs=x1[:], start=True, stop=True)
        x2t = cpool.tile([36, GPD], f32, tag="x2t")
        nc.scalar.activation(x2t[:], px2[:], mybir.ActivationFunctionType.Identity,
                             bias=b1[:])
        x2m = cpool.tile([36, GPD], f32, tag="x2m")
        nc.vector.tensor_scalar_mul(x2m[:], x2t[:], 0.01)
        x2 = cpool.tile([36, GPD], f32, tag="x2")
        nc.vector.tensor_tensor(out=x2[:], in0=x2t[:], in1=x2m[:],
                                op=mybir.AluOpType.max)
        px3 = ps1pool.tile([1, GPD], f32, tag="pstmp")
        nc.tensor.matmul(out=px3[:], lhsT=W2[:], rhs=x2[:], start=True, stop=True)
        x3 = cpool.tile([1, GPD], f32, tag="x3")
        nc.scalar.activation(x3[:], px3[:], mybir.ActivationFunctionType.Identity,
                             bias=b2[:])
        nc.sync.dma_start(t_out[:], x3[:])

    nc.compile()
    return nc


# ======================= entry point =======================

def kernel(Vnode, Vedge, y, W_in, b_in, Wself, Wnbr, bl, Wout, bout,
           W1, b1, W2, b2):
    cfg = _prep(Vnode, Vedge, y)
    key = (cfg["S"], cfg["B"])
    if key not in _CACHE:
        _CACHE[key] = _build(cfg)
    nc = _CACHE[key]

    f32 = np.float32
    shared = dict(
        W_in=np.ascontiguousarray(W_in, f32),
        b_in=np.asarray(b_in, f32).reshape(HID, 1),
        Wself=np.ascontiguousarray(Wself, f32),
        Wnbr=np.ascontiguousarray(Wnbr, f32),
        bl=np.asarray(bl, f32).reshape(N_LAYERS, HID, 1),
        Wout=np.ascontiguousarray(Wout, f32),
        bout=np.asarray(bout, f32).reshape(OUT_DIM, 1),
        W1=np.ascontiguousarray(W1, f32),
        b1=np.asarray(b1, f32).reshape(36, 1),
        W2=np.ascontiguousarray(W2, f32),
        b2=np.asarray(b2, f32).reshape(1, 1),
    )
    in_maps = []
    for d in range(N_CORES):
        in_maps.append(dict(
            vnode=cfg["vnode_dev"][d],
            idxs=cfg["idxs"][d],
            obuf=cfg["Obuf"][d],
            pmat=cfg["P"][d],
            invcnt=cfg["invcnt"][d],
            **shared))

    from concourse import bass_utils
    res = bass_utils.run_bass_kernel_spmd(nc, in_maps, core_ids=list(range(N_CORES)))
    out = np.concatenate([res.results[d]["out"].reshape(GPD) for d in range(N_CORES)])
    return out.reshape(N_GRAPHS, 1).astype(np.float32)
